# revision 5
# baseline (speedup 1.0000x reference)
"""Trainium2 Bass kernel for nn_Attention_7919919694519.

Multi-head attention (B=2, L=2048, H=16, d=64) with two data-dependent masks:
  - V_len[b] masks HEADS h >= V_len[b]: the reference adds -1e12 to every
    score of those heads, which collapses (in fp32) to a uniform softmax, so
    the masked head's output is mean_k(v) = (mean_k V_seq) @ WV_h  (rank-1).
  - Q_len[b] zeroes output rows q >= Q_len[b].

Strategy (host-visible Q_len/V_len drive the work list):
  - Only unmasked heads with live q rows do real attention. Work is split
    into units (b, h, q-chunk of 512 rows), dealt evenly to 8 NeuronCores
    (SPMD: same NEFF, different data). No collectives; host gathers.
  - Per unit on device: project q/k/v (bf16 matmuls from host-transposed
    inputs), scores S^T[k,q] = (K W_k)^T-tile^T @ (Q W_q/8)^T in PSUM,
    exp on ScalarE (PSUM->SBUF bf16), AV accumulation with a ones-column
    appended to v so the softmax denominators fall out of the same matmuls,
    PE transpose, reciprocal+scale on VectorE, DMA out.
  - Masked-head rank-1 content: device reduces V_seq over k (VectorE) and
    projects through WV/2048; host broadcasts rows (pure output assembly).
"""

import math
import numpy as np
import ml_dtypes

import concourse.bass as bass
import concourse.tile as tile
from concourse import bacc, mybir
from concourse.bass_utils import run_bass_kernel_spmd
from concourse.masks import make_identity
from contextlib import ExitStack

BF16 = ml_dtypes.bfloat16
N_CORES = 8
B_, L_, D_, H_ = 2, 2048, 64, 16
NQ = 512              # q rows per unit
KT = 16               # number of 128-row k tiles (L/128)
CHUNKS = [3, 3, 3, 3, 3, 1]   # k-tiles per score chunk (PSUM banks each)

_cache = {}


def _build(n_units):
    """Build + compile the SPMD NEFF for n_units attention units per core."""
    nc = bacc.Bacc("TRN2", target_bir_lowering=False, debug=False,
                   num_devices=N_CORES)
    dt = mybir.dt
    U = n_units

    qt_d = nc.dram_tensor("qt", [U, 64, NQ], dt.bfloat16, kind="ExternalInput").ap()
    kt_d = nc.dram_tensor("kt", [U, 64, L_], dt.bfloat16, kind="ExternalInput").ap()
    vt_d = nc.dram_tensor("vt", [U, 64, L_], dt.bfloat16, kind="ExternalInput").ap()
    w_d = nc.dram_tensor("w", [U, 64, 192], dt.bfloat16, kind="ExternalInput").ap()
    vtb_d = nc.dram_tensor("vtb", [B_, 64, L_], dt.float32, kind="ExternalInput").ap()
    wvm_d = nc.dram_tensor("wvm", [64, H_ * 64], dt.float32, kind="ExternalInput").ap()
    out_d = nc.dram_tensor("out", [U, NQ, 64], dt.float32, kind="ExternalOutput").ap()
    mo_d = nc.dram_tensor("meanout", [128, 8, B_], dt.float32, kind="ExternalOutput").ap()

    with tile.TileContext(nc) as tc, ExitStack() as ctx:
        inp = ctx.enter_context(tc.tile_pool(name="inp", bufs=2))
        proj = ctx.enter_context(tc.tile_pool(name="proj", bufs=2))
        expp = ctx.enter_context(tc.tile_pool(name="expp", bufs=3))
        ob = ctx.enter_context(tc.tile_pool(name="ob", bufs=2))
        single = ctx.enter_context(tc.tile_pool(name="single", bufs=1))
        ps_s = ctx.enter_context(tc.tile_pool(name="ps_s", bufs=2, space="PSUM"))
        ps_a = ctx.enter_context(tc.tile_pool(name="ps_a", bufs=2, space="PSUM"))

        ident = single.tile([128, 128], dt.float32)
        make_identity(nc, ident[:])

        # ---- masked-head rank-1 content: (sum_k V_seq) @ (WV/2048) ----
        wvm_sb = single.tile([64, H_ * 64], dt.float32)
        nc.sync.dma_start(wvm_sb[:], wvm_d[:])
        mvt = single.tile([64, B_], dt.float32)
        for b in range(B_):
            vtb_sb = inp.tile([64, L_], dt.float32, tag="vtb")
            nc.sync.dma_start(vtb_sb[:], vtb_d[b])
            nc.vector.reduce_sum(mvt[:, b:b + 1], vtb_sb[:], axis=mybir.AxisListType.X)
        mo_sb = single.tile([128, 8, B_], dt.float32)
        for c in range(8):
            mps = ps_a.tile([128, B_], dt.float32, tag="pa")
            nc.tensor.matmul(mps[:], wvm_sb[:, c * 128:(c + 1) * 128], mvt[:],
                             start=True, stop=True)
            nc.vector.tensor_copy(mo_sb[:, c, :], mps[:])
        nc.sync.dma_start(mo_d[:], mo_sb[:])

        # ---- attention units ----
        for u in range(U):
            qt_sb = inp.tile([64, NQ], dt.bfloat16, tag="qt")
            nc.sync.dma_start(qt_sb[:], qt_d[u])
            kt_sb = inp.tile([64, L_], dt.bfloat16, tag="kt")
            nc.sync.dma_start(kt_sb[:], kt_d[u])
            vt_sb = inp.tile([64, L_], dt.bfloat16, tag="vt")
            nc.sync.dma_start(vt_sb[:], vt_d[u])
            w_sb = inp.tile([64, 192], dt.bfloat16, tag="w")
            nc.sync.dma_start(w_sb[:], w_d[u])

            # q projection: qTh[64, NQ] = (WQ_h/8).T-form @ Q^T slice
            qps = ps_a.tile([64, NQ], dt.float32, tag="pa")
            nc.tensor.matmul(qps[:], w_sb[:, 0:64], qt_sb[:], start=True, stop=True)
            qTh = proj.tile([64, NQ], dt.bfloat16, tag="qTh")
            nc.vector.tensor_copy(qTh[:], qps[:])

            # k projection: kTh[64, L]
            kTh = proj.tile([64, L_], dt.bfloat16, tag="kTh")
            for half in range(2):
                kps = ps_s.tile([64, 1024], dt.float32, tag="ps")
                for j in range(2):
                    nc.tensor.matmul(kps[:, j * 512:(j + 1) * 512],
                                     w_sb[:, 64:128],
                                     kt_sb[:, half * 1024 + j * 512:
                                           half * 1024 + (j + 1) * 512],
                                     start=True, stop=True)
                nc.vector.tensor_copy(kTh[:, half * 1024:(half + 1) * 1024], kps[:])

            # v projection into [k=128, 16, 65] layout (col 64 = ones)
            v_sb = proj.tile([128, KT, 65], dt.bfloat16, tag="v_sb")
            for half in range(2):
                vps = ps_s.tile([128, 8 * 64], dt.float32, tag="ps")
                for j in range(8):
                    t = half * 8 + j
                    nc.tensor.matmul(vps[:, j * 64:(j + 1) * 64],
                                     vt_sb[:, t * 128:(t + 1) * 128],
                                     w_sb[:, 128:192], start=True, stop=True)
                nc.vector.tensor_copy(
                    v_sb[:, half * 8:(half + 1) * 8, 0:64],
                    vps[:].rearrange("p (t d) -> p t d", t=8))
            nc.vector.memset(v_sb[:, :, 64], 1.0)

            # scores + softmax-numerator + AV, chunked over k tiles
            av = ps_a.tile([65, NQ], dt.float32, tag="pa")
            t0 = 0
            for cl in CHUNKS:
                sps = ps_s.tile([128, cl * NQ], dt.float32, tag="ps")
                for j in range(cl):
                    t = t0 + j
                    nc.tensor.matmul(sps[:, j * NQ:(j + 1) * NQ],
                                     kTh[:, t * 128:(t + 1) * 128],
                                     qTh[:], start=True, stop=True)
                ex = expp.tile([128, cl * NQ], dt.bfloat16, tag="ex")
                nc.scalar.activation(ex[:], sps[:], mybir.ActivationFunctionType.Exp)
                for j in range(cl):
                    t = t0 + j
                    nc.tensor.matmul(av[:], v_sb[:, t, :],
                                     ex[:, j * NQ:(j + 1) * NQ],
                                     start=(t == 0), stop=(t == KT - 1))
                t0 += cl

            # normalize + transpose + store
            o_sb = ob.tile([65, NQ], dt.float32, tag="o_sb")
            nc.vector.tensor_copy(o_sb[:], av[:])
            for j in range(NQ // 128):
                tp = ps_a.tile([128, 65], dt.float32, tag="pa")
                nc.tensor.transpose(tp[:], o_sb[:, j * 128:(j + 1) * 128],
                                    ident[0:65, 0:65])
                rs = ob.tile([128, 1], dt.float32, tag="rs")
                nc.vector.reciprocal(rs[:], tp[:, 64:65])
                ot = ob.tile([128, 64], dt.float32, tag="ot")
                nc.vector.tensor_scalar_mul(ot[:], tp[:, 0:64], rs[:])
                nc.sync.dma_start(out_d[u, j * 128:(j + 1) * 128, :], ot[:])

    nc.compile()
    return nc


def kernel(Q_seq, K_seq, V_seq, WQ, WK, WV, Q_len, V_len):
    Q_seq = np.asarray(Q_seq, dtype=np.float32)
    K_seq = np.asarray(K_seq, dtype=np.float32)
    V_seq = np.asarray(V_seq, dtype=np.float32)
    WQ = np.asarray(WQ, dtype=np.float32)
    WK = np.asarray(WK, dtype=np.float32)
    WV = np.asarray(WV, dtype=np.float32)
    q_len = [int(x) for x in np.asarray(Q_len).reshape(-1)]
    v_len = [int(x) for x in np.asarray(V_len).reshape(-1)]
    B, L, d = Q_seq.shape
    H = WQ.shape[1] // d
    scale = 1.0 / math.sqrt(d)

    # work list: (b, h, q0) for unmasked heads, live q chunks
    units = []
    for b in range(B):
        nq = min(max(q_len[b], 0), L)
        nh = min(max(v_len[b], 0), H)
        for h in range(nh):
            for q0 in range(0, nq, NQ):
                units.append((b, h, q0))
    n_real = len(units)
    U = max(1, (n_real + N_CORES - 1) // N_CORES)
    dummy = units[-1] if units else (0, 0, 0)
    units_p = units + [dummy] * (N_CORES * U - n_real)

    if U not in _cache:
        _cache[U] = _build(U)
    nc = _cache[U]

    # host-side shard prep (transposes, bf16 casts, weight slicing)
    KTb = [np.ascontiguousarray(K_seq[b].T).astype(BF16) for b in range(B)]
    VTb = [np.ascontiguousarray(V_seq[b].T).astype(BF16) for b in range(B)]
    QT = [np.ascontiguousarray(Q_seq[b].T).astype(BF16) for b in range(B)]
    zeros_w = np.zeros((64, 192), dtype=BF16)
    vtb = np.stack([V_seq[b].T for b in range(B)]).astype(np.float32)
    wvm = (WV / float(L)).astype(np.float32)

    in_maps = []
    for c in range(N_CORES):
        qt = np.empty((U, 64, NQ), dtype=BF16)
        kt = np.empty((U, 64, L), dtype=BF16)
        vt = np.empty((U, 64, L), dtype=BF16)
        w = np.empty((U, 64, 192), dtype=BF16)
        for j in range(U):
            b, h, q0 = units_p[c * U + j]
            qt[j] = QT[b][:, q0:q0 + NQ]
            kt[j] = KTb[b]
            vt[j] = VTb[b]
            if n_real == 0:
                w[j] = zeros_w
            else:
                w[j, :, 0:64] = (WQ[:, h * d:(h + 1) * d] * scale).astype(BF16)
                w[j, :, 64:128] = WK[:, h * d:(h + 1) * d].astype(BF16)
                w[j, :, 128:192] = WV[:, h * d:(h + 1) * d].astype(BF16)
        in_maps.append({"qt": qt, "kt": kt, "vt": vt, "w": w,
                        "vtb": vtb, "wvm": wvm})

    global _last_in_maps
    _last_in_maps = in_maps
    res = run_bass_kernel_spmd(nc, in_maps, core_ids=list(range(N_CORES)))
    results = res.results

    # gather
    out = np.zeros((B, L, H * d), dtype=np.float32)
    mo = results[0]["meanout"]  # [128, 8, B]
    mean_proj = np.transpose(mo, (2, 1, 0)).reshape(B, H * d)  # [B, 1024]
    for b in range(B):
        nq = min(max(q_len[b], 0), L)
        nh = min(max(v_len[b], 0), H)
        if nq > 0 and nh < H:
            out[b, :nq, nh * d:] = mean_proj[b, nh * d:][None, :]
    for idx, (b, h, q0) in enumerate(units):
        c, j = divmod(idx, U)
        live = min(NQ, min(max(q_len[b], 0), L) - q0)
        out[b, q0:q0 + live, h * d:(h + 1) * d] = results[c]["out"][j, :live, :]
    return out
